# revision 22
# baseline (speedup 1.0000x reference)
"""Bass/Trainium2 kernel for nn_DynamicsNet (gnn_message_passing).

Sharding: pure data parallel over batch, 8 cores x 256 samples.

Device design (per core, B_LOC=256 samples, chunks of CH=16):
  - Activations in SBUF fp16; PSUM accumulates fp32.
  - Two per-sample layouts:
      nm (node-major):    [n=96 partitions, d=128 free]  -> LN / mish / residual
      fm (feature-major): [d=128 partitions, n=96 free]  -> conv intermediate
  - A-step (graph conv):  out_b = x_b.T @ A        per-sample matmul, nm -> fm
  - W-step (1x1 conv):    out_b = msg_b.T @ w.T    per-sample matmul, fm -> nm
    (the layout flip rides the matmul: the activation is the stationary operand)
  - conv bias: added during PSUM->SBUF evacuation (TT add, bcast bias tile)
  - LN stats: chunked 3D bn_stats, manual even/odd aggregation, half-layer
    (128-sample) granularity for the tiny derive ops.
  - mish: Exp -> Ln(1+x) -> Tanh ACT chain + DVE mul (native Mish/Softplus
    LUTs are broken on this toolchain).
  - final: per-sample PE transpose (nm -> output [d, n]) + min-max scaling
    fused into the PSUM evacuation; min/max via DVE reduce + GPSIMD
    partition_all_reduce.
"""

import os

import numpy as np

N_NODES, D_MODEL, N_BLOCKS = 96, 128, 8
B_FULL = 2048
N_CORES = 8
B_LOC = B_FULL // N_CORES          # 256
CH = 16                            # samples per chunk
N_CH = B_LOC // CH                 # 16 chunks
HALF = B_LOC // 2                  # 128 samples per stats half
LOGITS = 401
EPS = 1e-5

_CACHED = {}
_PATCHED = [False]


def _patch_toolchain():
    """One-time environment fixups (idempotent):

    1. walrus in this container encodes at most ONE sync-wait per ISA
       instruction ("Too many sync wait commands"); Tile emits multi-wait
       instructions. Legalize at BIR-serialization time by hoisting extra
       waits into preceding same-engine NoOps (engine FIFO makes this
       semantically identical).
    2. The PWP act tables ship a mish LUT but act_info.json omits "mish"
       from the mish_and_others set, so lower_act rejects
       ActivationFunctionType.Mish. Patch the json (nix store is writable
       here); also point BASS_ACT_ROOT_JSON_PATH at it for the direct path.
    """
    if _PATCHED[0]:
        return
    _PATCHED[0] = True

    import json as _json

    import concourse.bass as bass

    orig = bass.Bass.to_json_bytes

    def to_json_bytes_legal(self, *a, **k):
        d = _json.loads(orig(self, *a, **k))
        ctr = 0
        for fn in d.get("functions", []):
            for blk in fn.get("blocks", []):
                out = []
                for inst in blk.get("instructions", []):
                    si = inst.get("sync_info")
                    if si:
                        waits = si.get("on_wait") or []
                        if len(waits) > 1:
                            for w in waits[:-1]:
                                ctr += 1
                                out.append({
                                    "debug": inst.get("debug", 0),
                                    "engine": inst["engine"],
                                    "ins": [], "outs": [],
                                    "name": f"I-syncw{ctr}",
                                    "opcode": "NoOp",
                                    "sync_info": {"on_wait": [w],
                                                  "on_update": []},
                                })
                            si["on_wait"] = waits[-1:]
                    out.append(inst)
                blk["instructions"] = out
        return _json.dumps(d).encode()

    bass.Bass.to_json_bytes = to_json_bytes_legal

    # --- act tables: enable mish ---
    try:
        from neuronxcc.driver.Job import Job
        from neuronxcc.driver.jobs.support.FindActInfo import findActInfoFile
        p = findActInfoFile(Job.getPackageDir(), "gen3")
        with open(p) as f:
            d = _json.load(f)
        changed = False
        for s in d.get("act_func_sets", []):
            if s["name"] == "mish_and_others" and "mish" not in s["act"]:
                s["act"]["mish"] = 4
                changed = True
            if s["name"] == "softplus_and_others" and "softplus" not in s["act"]:
                s["act"]["softplus"] = 4
                changed = True
        if changed:
            with open(p, "w") as f:
                _json.dump(d, f)
        os.environ.setdefault("BASS_ACT_ROOT_JSON_PATH", p)
    except Exception:
        pass


def _build_bass(use_native_mish: bool, apply_gb: bool, apply_rngb: bool):
    import concourse.bass as bass
    import concourse.bass_isa as bass_isa
    import concourse.mybir as mybir
    import concourse.tile as tile

    f16 = mybir.dt.float16
    f32 = mybir.dt.float32
    AF = mybir.ActivationFunctionType
    OP = mybir.AluOpType
    AX = mybir.AxisListType

    nc = bass.Bass()

    # ---------------- DRAM tensors ----------------
    h_in = nc.dram_tensor("h16", [B_LOC, D_MODEL, N_NODES], f16, kind="ExternalInput")
    qpb_in = nc.dram_tensor("qpb", [B_LOC, D_MODEL], f16, kind="ExternalInput")
    aembT_in = nc.dram_tensor("aembT", [D_MODEL, B_LOC], f16, kind="ExternalInput")
    A_in = nc.dram_tensor("A96", [N_NODES, N_NODES], f16, kind="ExternalInput")
    I96_in = nc.dram_tensor("I96", [N_NODES, N_NODES], f16, kind="ExternalInput")
    I128_in = nc.dram_tensor("I128", [128, 128], f16, kind="ExternalInput")
    wAT_in = nc.dram_tensor("wAT", [D_MODEL, D_MODEL], f16, kind="ExternalInput")
    Vt_in = nc.dram_tensor("Vt", [16, D_MODEL, D_MODEL], f16, kind="ExternalInput")
    cbb_in = nc.dram_tensor("cbb", [16, D_MODEL], f16, kind="ExternalInput")
    lng_in = nc.dram_tensor("lng", [16, D_MODEL], f16, kind="ExternalInput")
    lnb_in = nc.dram_tensor("lnb", [16, D_MODEL], f16, kind="ExternalInput")
    r1wT0_in = nc.dram_tensor("r1wT0", [D_MODEL, 64], f16, kind="ExternalInput")
    r1wT1_in = nc.dram_tensor("r1wT1", [D_MODEL, 64], f16, kind="ExternalInput")
    r1b_in = nc.dram_tensor("r1b", [1, 64], f16, kind="ExternalInput")
    r2wT_in = nc.dram_tensor("r2wT", [64, LOGITS], f16, kind="ExternalInput")
    r2b_in = nc.dram_tensor("r2b", [1, LOGITS], f16, kind="ExternalInput")
    rng_in = nc.dram_tensor("rng", [1, 64], f16, kind="ExternalInput")
    rnb_in = nc.dram_tensor("rnb", [1, 64], f16, kind="ExternalInput")

    h_out = nc.dram_tensor("h_out", [B_LOC, D_MODEL, N_NODES], f32,
                           kind="ExternalOutput")
    lg_out = nc.dram_tensor("lg_out", [B_LOC, LOGITS], f32, kind="ExternalOutput")

    SQ128 = float(np.sqrt(128.0))

    def bcast_row(dram_ap, parts, free):
        """AP reading one DRAM row broadcast across `parts` partitions."""
        return bass.AP(tensor=dram_ap.tensor, offset=dram_ap.offset,
                       ap=[[0, parts], [1, free]])

    with tile.TileContext(nc) as tc:
        with (
            tc.tile_pool(name="const", bufs=1) as cpool,
            tc.tile_pool(name="xres", bufs=1) as xpool,
            tc.tile_pool(name="zbuf", bufs=10) as zpool,
            tc.tile_pool(name="htile", bufs=2) as hpool,
            tc.tile_pool(name="msg", bufs=2) as mpool,
            tc.tile_pool(name="ubuf", bufs=1) as upool,
            tc.tile_pool(name="ybuf", bufs=8) as ypool,
            tc.tile_pool(name="stats", bufs=2) as spool,
            tc.tile_pool(name="small", bufs=2) as smpool,
            tc.tile_pool(name="once", bufs=1) as onpool,
            tc.tile_pool(name="ostage", bufs=1) as opool,
            tc.tile_pool(name="dscr", bufs=1, space="DRAM") as dpool,
            tc.tile_pool(name="psA", bufs=1, space="PSUM") as psA,
            tc.tile_pool(name="psW", bufs=1, space="PSUM") as psW,
        ):
            # ------------- constants -------------
            A_sb = cpool.tile([N_NODES, N_NODES], f16)
            nc.sync.dma_start(out=A_sb, in_=A_in[:, :])
            I96_sb = cpool.tile([N_NODES, N_NODES], f16)
            nc.sync.dma_start(out=I96_sb, in_=I96_in[:, :])
            I128_sb = cpool.tile([128, 128], f16)
            nc.sync.dma_start(out=I128_sb, in_=I128_in[:, :])
            wAT_sb = cpool.tile([D_MODEL, D_MODEL], f16)
            nc.sync.dma_start(out=wAT_sb, in_=wAT_in[:, :])
            V_sb = cpool.tile([D_MODEL, 16, D_MODEL], f16)
            for li in range(16):
                nc.sync.dma_start(out=V_sb[:, li, :], in_=Vt_in[li, :, :])
            cb_sb = cpool.tile([N_NODES, 16, D_MODEL], f16)
            for li in range(16):
                nc.sync.dma_start(out=cb_sb[:, li, :],
                                  in_=bcast_row(cbb_in[li, :], N_NODES, D_MODEL))
            if apply_gb:
                g_sb = cpool.tile([N_NODES, 16, D_MODEL], f16)
                b_sb = cpool.tile([N_NODES, 16, D_MODEL], f16)
                for li in range(16):
                    nc.sync.dma_start(out=g_sb[:, li, :],
                                      in_=bcast_row(lng_in[li, :], N_NODES, D_MODEL))
                    nc.sync.dma_start(out=b_sb[:, li, :],
                                      in_=bcast_row(lnb_in[li, :], N_NODES, D_MODEL))

            aembT_sb = cpool.tile([D_MODEL, B_LOC], f16)
            nc.sync.dma_start(out=aembT_sb, in_=aembT_in[:, :])
            r1wT0_sb = cpool.tile([D_MODEL, 64], f16)
            nc.sync.dma_start(out=r1wT0_sb, in_=r1wT0_in[:, :])
            r1wT1_sb = cpool.tile([D_MODEL, 64], f16)
            nc.sync.dma_start(out=r1wT1_sb, in_=r1wT1_in[:, :])
            r1b_sb = cpool.tile([1, 64], f16)
            nc.sync.dma_start(out=r1b_sb, in_=r1b_in[:, :])
            r2wT_sb = cpool.tile([64, LOGITS], f16)
            nc.sync.dma_start(out=r2wT_sb, in_=r2wT_in[:, :])
            r2b_sb = cpool.tile([1, LOGITS], f16)
            nc.sync.dma_start(out=r2b_sb, in_=r2b_in[:, :])
            if apply_rngb:
                rng_sb = cpool.tile([128, 64], f16)
                rnb_sb = cpool.tile([128, 64], f16)
                nc.sync.dma_start(out=rng_sb, in_=bcast_row(rng_in[0, :], 128, 64))
                nc.sync.dma_start(out=rnb_sb, in_=bcast_row(rnb_in[0, :], 128, 64))
            ones_sb = cpool.tile([1, B_LOC], f16)
            nc.vector.memset(ones_sb, 1.0)
            eps_sb = cpool.tile([N_NODES, 1], f32)
            nc.vector.memset(eps_sb, 128.0 * EPS)
            eps64_sb = cpool.tile([128, 1], f32)
            nc.vector.memset(eps64_sb, 64.0 * EPS)
            one96_sb = cpool.tile([N_NODES, 1], f32)
            nc.vector.memset(one96_sb, 1.0)

            x_sb = xpool.tile([N_NODES, B_LOC, D_MODEL], f16)
            hsumf = cpool.tile([D_MODEL, B_LOC], f32)
            hsum16 = cpool.tile([D_MODEL, B_LOC], f16)
            pmax_sb = cpool.tile([N_NODES, B_LOC], f16)
            pmin_sb = cpool.tile([N_NODES, B_LOC], f16)

            # ---------------- proj ----------------
            for c in range(N_CH):
                b0 = c * CH
                ht = hpool.tile([D_MODEL, CH, N_NODES], f16, tag="ht")
                nc.sync.dma_start(
                    out=ht, in_=h_in[b0:b0 + CH, :, :].rearrange("b c n -> c b n"))
                nc.vector.tensor_reduce(out=hsumf[:, b0:b0 + CH], in_=ht,
                                        axis=AX.X, op=OP.add)
                # qpb rows broadcast across the 96 node partitions
                qsl = qpb_in[b0:b0 + CH, :]
                qb = mpool.tile([N_NODES, CH, D_MODEL], f16, tag="msg")
                nc.sync.dma_start(
                    out=qb,
                    in_=bass.AP(tensor=qsl.tensor, offset=qsl.offset,
                                ap=[[0, N_NODES], [D_MODEL, CH], [1, D_MODEL]]))
                psx = psW.tile([N_NODES, CH, D_MODEL], f32, tag="W")
                for j in range(CH):
                    nc.tensor.matmul(psx[:, j, :], ht[:, j, :], wAT_sb,
                                     start=True, stop=True)
                nc.vector.tensor_tensor(out=x_sb[:, b0:b0 + CH, :], in0=psx,
                                        in1=qb, op=OP.add)
            nc.vector.tensor_copy(hsum16, hsumf)

            # ---------------- blocks ----------------
            def half_block(l_idx, src_of, with_mish, y_chunks):
                """src nm -> A -> W(+bias) -> LN (+mish into y | +residual x).

                src_of(c) -> [96, CH, 128] tile/AP for chunk c.
                with_mish: writes mish output into fresh y_chunks[c] tiles.
                else: residual-adds into x_sb.
                """
                for hf in range(2):
                    zch = [None] * (N_CH // 2)
                    stats6 = spool.tile([N_NODES, HALF, 6], f32, tag="st6")
                    hb0 = hf * HALF
                    for c in range(N_CH // 2):
                        b0 = hb0 + c * CH
                        zb0 = c * CH
                        srct = src_of(hf * (N_CH // 2) + c)
                        # pad per-sample stride to 512B so each matmul
                        # output stays inside one PSUM bank
                        psa = psA.tile([D_MODEL, CH, 128], f32, tag="A")
                        for j in range(CH):
                            nc.tensor.matmul(psa[:, j, 0:N_NODES], srct[:, j, :],
                                             A_sb, start=True, stop=True)
                        msg = mpool.tile([D_MODEL, CH, N_NODES], f16, tag="msg")
                        nc.scalar.copy(out=msg, in_=psa[:, :, 0:N_NODES])
                        psw = psW.tile([N_NODES, CH, D_MODEL], f32, tag="W")
                        for j in range(CH):
                            nc.tensor.matmul(psw[:, j, :], msg[:, j, :],
                                             V_sb[:, l_idx, :], start=True,
                                             stop=True)
                        cb_bc = bass.AP(
                            tensor=cb_sb.tensor,
                            offset=cb_sb[:, l_idx, :].offset,
                            ap=[list(cb_sb[:, l_idx, :].ap[0]),
                                [0, CH], [1, D_MODEL]])
                        zt = zpool.tile([N_NODES, CH, D_MODEL], f16, tag="z")
                        zch[c] = zt
                        nc.vector.tensor_tensor(out=zt, in0=psw, in1=cb_bc,
                                                op=OP.add)
                        for j in range(CH):  # walrus: bn_stats out = 6/partition
                            nc.vector.bn_stats(out=stats6[:, zb0 + j, :],
                                               in_=zt[:, j, :])
                    # derive mean (mm) and rstd (rr) for this half
                    s1 = stats6[:, :, 1]
                    s2 = stats6[:, :, 2]
                    s4 = stats6[:, :, 4]
                    s5 = stats6[:, :, 5]
                    mm = smpool.tile([N_NODES, HALF], f32, tag="mm")
                    nc.vector.tensor_tensor(out=mm, in0=s1, in1=s4, op=OP.add)
                    nc.vector.tensor_scalar_mul(mm, mm, 0.5)
                    dlt = smpool.tile([N_NODES, HALF], f32, tag="dlt")
                    nc.vector.tensor_tensor(out=dlt, in0=s1, in1=s4,
                                            op=OP.subtract)
                    tv = smpool.tile([N_NODES, HALF], f32, tag="tv")
                    nc.vector.tensor_tensor(out=tv, in0=dlt, in1=dlt, op=OP.mult)
                    nc.vector.scalar_tensor_tensor(out=tv, in0=tv, scalar=32.0,
                                                   in1=s2, op0=OP.mult, op1=OP.add)
                    nc.vector.tensor_tensor(out=tv, in0=tv, in1=s5, op=OP.add)
                    nc.scalar.activation(out=tv, in_=tv, func=AF.Sqrt,
                                         bias=eps_sb, scale=1.0)
                    rr = smpool.tile([N_NODES, HALF], f32, tag="rr")
                    nc.vector.reciprocal(out=rr, in_=tv)
                    nc.vector.tensor_scalar_mul(rr, rr, SQ128)
                    # apply (+ mish / + residual)
                    for c in range(N_CH // 2):
                        b0 = hf * HALF + c * CH
                        zb0 = c * CH
                        u = upool.tile([N_NODES, CH, D_MODEL], f16, tag="u")
                        zt = zch[c]
                        for j in range(CH):
                            zb = zb0 + j
                            nc.vector.tensor_scalar(
                                out=u[:, j, :], in0=zt[:, j, :],
                                scalar1=mm[:, zb:zb + 1], scalar2=rr[:, zb:zb + 1],
                                op0=OP.subtract, op1=OP.mult)
                        if apply_gb:
                            g_bc = bass.AP(
                                tensor=g_sb.tensor, offset=g_sb[:, l_idx, :].offset,
                                ap=[list(g_sb[:, l_idx, :].ap[0]),
                                    [0, CH], [1, D_MODEL]])
                            b_bc = bass.AP(
                                tensor=b_sb.tensor, offset=b_sb[:, l_idx, :].offset,
                                ap=[list(b_sb[:, l_idx, :].ap[0]),
                                    [0, CH], [1, D_MODEL]])
                            nc.vector.tensor_tensor(out=u, in0=u, in1=g_bc,
                                                    op=OP.mult)
                            nc.vector.tensor_tensor(out=u, in0=u, in1=b_bc,
                                                    op=OP.add)
                        if with_mish:
                            yt = ypool.tile([N_NODES, CH, D_MODEL], f16, tag="y")
                            y_chunks[hf * (N_CH // 2) + c] = yt
                            if use_native_mish:
                                nc.scalar.activation(out=yt, in_=u, func=AF.Mish)
                            else:
                                ew = upool.tile([N_NODES, CH, D_MODEL], f16,
                                                tag="ew")
                                nc.scalar.activation(out=ew, in_=u, func=AF.Exp)
                                nc.scalar.activation(out=ew, in_=ew, func=AF.Ln,
                                                     bias=one96_sb, scale=1.0)
                                nc.scalar.activation(out=ew, in_=ew, func=AF.Tanh)
                                nc.vector.tensor_tensor(out=yt, in0=u, in1=ew,
                                                        op=OP.mult)
                        else:
                            nc.gpsimd.tensor_tensor(
                                out=x_sb[:, b0:b0 + CH, :],
                                in0=x_sb[:, b0:b0 + CH, :], in1=u, op=OP.add)

            def x_of(c):
                return x_sb[:, c * CH:(c + 1) * CH, :]

            for blk in range(N_BLOCKS):
                y_chunks = [None] * N_CH
                half_block(2 * blk + 0, x_of, True, y_chunks)
                half_block(2 * blk + 1, lambda c: y_chunks[c], False, None)

            # ---------------- min/max + final transpose/scale ----------------
            for c in range(N_CH):
                b0 = c * CH
                nc.vector.tensor_reduce(out=pmax_sb[:, b0:b0 + CH],
                                        in_=x_sb[:, b0:b0 + CH, :], axis=AX.X,
                                        op=OP.max)
                nc.vector.tensor_reduce(out=pmin_sb[:, b0:b0 + CH],
                                        in_=x_sb[:, b0:b0 + CH, :], axis=AX.X,
                                        op=OP.min)
            # cross-partition (96 -> 1) min/max via PE transposes
            mxT = onpool.tile([128, 2], f16, tag="mxT")
            mnT = onpool.tile([128, 2], f16, tag="mnT")
            for hf in range(2):
                psT = psW.tile([128, N_NODES], f16, tag="W")
                nc.tensor.matmul(psT, pmax_sb[:, hf * 128:(hf + 1) * 128],
                                 I96_sb, start=True, stop=True, is_transpose=True)
                nc.vector.tensor_reduce(out=mxT[:, hf:hf + 1], in_=psT,
                                        axis=AX.X, op=OP.max)
                psT2 = psW.tile([128, N_NODES], f16, tag="W")
                nc.tensor.matmul(psT2, pmin_sb[:, hf * 128:(hf + 1) * 128],
                                 I96_sb, start=True, stop=True, is_transpose=True)
                nc.vector.tensor_reduce(out=mnT[:, hf:hf + 1], in_=psT2,
                                        axis=AX.X, op=OP.min)
            # transpose [128, 2] -> [2, 128], evac, DMA-broadcast to all parts
            ps2a = psW.tile([2, 128], f16, tag="W")
            nc.tensor.matmul(ps2a, mxT, I128_sb, start=True, stop=True,
                             is_transpose=True)
            mx1p = onpool.tile([2, 128], f16, tag="mx1p")
            nc.vector.tensor_copy(mx1p, ps2a)
            ps2b = psW.tile([2, 128], f16, tag="W")
            nc.tensor.matmul(ps2b, mnT, I128_sb, start=True, stop=True,
                             is_transpose=True)
            mn1p = onpool.tile([2, 128], f16, tag="mn1p")
            nc.vector.tensor_copy(mn1p, ps2b)
            mxd = dpool.tile([2, 128], f16, tag="mxd")
            mnd = dpool.tile([2, 128], f16, tag="mnd")
            nc.sync.dma_start(out=mxd, in_=mx1p)
            nc.sync.dma_start(out=mnd, in_=mn1p)
            mx_b16 = onpool.tile([128, 2, 128], f16, tag="mxa")
            mn_b16 = onpool.tile([128, 2, 128], f16, tag="mna")
            for j in range(2):
                nc.sync.dma_start(out=mx_b16[:, j, :],
                                  in_=bcast_row(mxd[j, :], 128, 128))
                nc.sync.dma_start(out=mn_b16[:, j, :],
                                  in_=bcast_row(mnd[j, :], 128, 128))
            mn_all = onpool.tile([128, B_LOC], f32, tag="mnf")
            nc.vector.tensor_copy(mn_all, mn_b16.rearrange("p a b -> p (a b)"))
            sc = onpool.tile([128, B_LOC], f32, tag="sc")
            nc.vector.tensor_tensor(out=sc,
                                    in0=mx_b16.rearrange("p a b -> p (a b)"),
                                    in1=mn_all, op=OP.subtract)
            lt = onpool.tile([128, B_LOC], f32, tag="lt")
            nc.vector.tensor_scalar(out=lt, in0=sc, scalar1=1e-5, scalar2=1e-5,
                                    op0=OP.is_lt, op1=OP.mult)
            nc.vector.tensor_tensor(out=sc, in0=sc, in1=lt, op=OP.add)
            rs_all = onpool.tile([128, B_LOC], f32, tag="rsa")
            nc.vector.reciprocal(out=rs_all, in_=sc)

            for c in range(N_CH):
                b0 = c * CH
                pso = psA.tile([D_MODEL, CH, 128], f16, tag="A")
                for j in range(CH):
                    nc.tensor.matmul(pso[:, j, 0:N_NODES], x_sb[:, b0 + j, :],
                                     I96_sb, start=True, stop=True,
                                     is_transpose=True)
                ost = opool.tile([D_MODEL, CH, N_NODES], f32, tag="ost")
                for j in range(CH):
                    b = b0 + j
                    nc.vector.tensor_scalar(
                        out=ost[:, j, :], in0=pso[:, j, 0:N_NODES],
                        scalar1=mn_all[:, b:b + 1], scalar2=rs_all[:, b:b + 1],
                        op0=OP.subtract, op1=OP.mult)
                nc.sync.dma_start(
                    out=h_out[b0:b0 + CH, :, :].rearrange("b c n -> c b n"),
                    in_=ost)

            # ---------------- reward head ----------------
            psr = psW.tile([64, B_LOC], f32, tag="W")
            nc.tensor.matmul(psr, r1wT0_sb, hsum16, start=True, stop=False)
            nc.tensor.matmul(psr, r1wT1_sb, aembT_sb, start=False, stop=False)
            nc.tensor.matmul(psr, r1b_sb, ones_sb, start=False, stop=True)
            rIn = onpool.tile([64, B_LOC], f16, tag="rin")
            nc.scalar.copy(out=rIn, in_=psr)
            rT = onpool.tile([128, 2, 64], f16, tag="rT")
            for hf in range(2):
                pst = psW.tile([128, 64], f16, tag="W")
                nc.tensor.matmul(pst, rIn[:, hf * 128:(hf + 1) * 128],
                                 I128_sb[0:64, 0:64], start=True, stop=True,
                                 is_transpose=True)
                nc.vector.tensor_copy(rT[:, hf, :], pst)
            rst6 = onpool.tile([128, 2, 6], f32, tag="rst6")
            for hf in range(2):
                nc.vector.bn_stats(out=rst6[:, hf, :], in_=rT[:, hf, :])
            rmm = onpool.tile([128, 2], f32, tag="rmm")
            nc.vector.tensor_tensor(out=rmm, in0=rst6[:, :, 1], in1=rst6[:, :, 4],
                                    op=OP.add)
            nc.vector.tensor_scalar_mul(rmm, rmm, 0.5)
            rdl = onpool.tile([128, 2], f32, tag="rdl")
            nc.vector.tensor_tensor(out=rdl, in0=rst6[:, :, 1], in1=rst6[:, :, 4],
                                    op=OP.subtract)
            rtv = onpool.tile([128, 2], f32, tag="rtv")
            nc.vector.tensor_tensor(out=rtv, in0=rdl, in1=rdl, op=OP.mult)
            nc.vector.scalar_tensor_tensor(out=rtv, in0=rtv, scalar=16.0,
                                           in1=rst6[:, :, 2], op0=OP.mult,
                                           op1=OP.add)
            nc.vector.tensor_tensor(out=rtv, in0=rtv, in1=rst6[:, :, 5], op=OP.add)
            nc.scalar.activation(out=rtv, in_=rtv, func=AF.Sqrt, bias=eps64_sb,
                                 scale=1.0)
            rrr = onpool.tile([128, 2], f32, tag="rrr")
            nc.vector.reciprocal(out=rrr, in_=rtv)
            nc.vector.tensor_scalar_mul(rrr, rrr, 8.0)
            for hf in range(2):
                nc.vector.tensor_scalar(
                    out=rT[:, hf, :], in0=rT[:, hf, :],
                    scalar1=rmm[:, hf:hf + 1], scalar2=rrr[:, hf:hf + 1],
                    op0=OP.subtract, op1=OP.mult)
                if apply_rngb:
                    nc.vector.tensor_tensor(out=rT[:, hf, :], in0=rT[:, hf, :],
                                            in1=rng_sb, op=OP.mult)
                    nc.vector.tensor_tensor(out=rT[:, hf, :], in0=rT[:, hf, :],
                                            in1=rnb_sb, op=OP.add)
            if use_native_mish:
                nc.scalar.activation(out=rT, in_=rT, func=AF.Mish)
            else:
                rew = onpool.tile([128, 2, 64], f16, tag="rew")
                rone = onpool.tile([128, 1], f32, tag="rone")
                nc.vector.memset(rone, 1.0)
                nc.scalar.activation(out=rew, in_=rT, func=AF.Exp)
                nc.scalar.activation(out=rew, in_=rew, func=AF.Ln, bias=rone,
                                     scale=1.0)
                nc.scalar.activation(out=rew, in_=rew, func=AF.Tanh)
                nc.vector.tensor_tensor(out=rT, in0=rT, in1=rew, op=OP.mult)
            lstage = onpool.tile([128, 2, LOGITS], f32, tag="lst")
            for hf in range(2):
                pstb = psW.tile([64, 128], f16, tag="W")
                nc.tensor.matmul(pstb, rT[:, hf, :], I128_sb, start=True,
                                 stop=True, is_transpose=True)
                rK = onpool.tile([64, 128], f16, tag="rK")
                nc.vector.tensor_copy(rK, pstb)
                psl = psW.tile([128, LOGITS], f32, tag="W")
                nc.tensor.matmul(psl, rK, r2wT_sb, start=True, stop=False)
                nc.tensor.matmul(psl, ones_sb[:, 0:128], r2b_sb, start=False,
                                 stop=True)
                nc.scalar.copy(out=lstage[:, hf, :], in_=psl)
                nc.sync.dma_start(out=lg_out[hf * 128:(hf + 1) * 128, :],
                                  in_=lstage[:, hf, :])

    return nc


def _host_prep(inputs):
    h = np.asarray(inputs["h"], np.float32)
    a = np.asarray(inputs["a"]).astype(np.int64)
    piece_id = np.asarray(inputs["piece_id"]).astype(np.int64)
    A_norm = np.asarray(inputs["A_norm"], np.float32)
    piece_emb = np.asarray(inputs["piece_emb"], np.float32)
    pos_emb = np.asarray(inputs["pos_emb"], np.float32)
    proj_w = np.asarray(inputs["proj_w"], np.float32)
    proj_b = np.asarray(inputs["proj_b"], np.float32)
    conv_w = np.asarray(inputs["conv_w"], np.float32)
    conv_b = np.asarray(inputs["conv_b"], np.float32)
    ln_g = np.asarray(inputs["ln_g"], np.float32)
    ln_b = np.asarray(inputs["ln_b"], np.float32)
    r1_w = np.asarray(inputs["r1_w"], np.float32)
    r1_b = np.asarray(inputs["r1_b"], np.float32)
    rn_g = np.asarray(inputs["rn_g"], np.float32)
    rn_b = np.asarray(inputs["rn_b"], np.float32)
    r2_w = np.asarray(inputs["r2_w"], np.float32)
    r2_b = np.asarray(inputs["r2_b"], np.float32)

    a_emb = piece_emb[piece_id] + pos_emb[a % N_NODES]
    qpb = a_emb @ proj_w[:, D_MODEL:].T + proj_b

    apply_gb = not (np.allclose(ln_g, 1.0) and np.allclose(ln_b, 0.0))
    apply_rngb = not (np.allclose(rn_g, 1.0) and np.allclose(rn_b, 0.0))

    shared = {
        "A96": A_norm.astype(np.float16),
        "I96": np.eye(N_NODES, dtype=np.float16),
        "I128": np.eye(128, dtype=np.float16),
        "wAT": np.ascontiguousarray(proj_w[:, :D_MODEL].T).astype(np.float16),
        "Vt": np.ascontiguousarray(
            conv_w.reshape(16, D_MODEL, D_MODEL).swapaxes(1, 2)).astype(np.float16),
        "cbb": conv_b.reshape(16, D_MODEL).astype(np.float16),
        "lng": ln_g.reshape(16, D_MODEL).astype(np.float16),
        "lnb": ln_b.reshape(16, D_MODEL).astype(np.float16),
        "r1wT0": np.ascontiguousarray(
            (r1_w[:, :D_MODEL] / float(N_NODES)).T).astype(np.float16),
        "r1wT1": np.ascontiguousarray(r1_w[:, D_MODEL:].T).astype(np.float16),
        "r1b": r1_b.reshape(1, 64).astype(np.float16),
        "r2wT": np.ascontiguousarray(r2_w.T).astype(np.float16),
        "r2b": r2_b.reshape(1, LOGITS).astype(np.float16),
        "rng": rn_g.reshape(1, 64).astype(np.float16),
        "rnb": rn_b.reshape(1, 64).astype(np.float16),
    }
    in_maps = []
    for s in range(N_CORES):
        sl = slice(s * B_LOC, (s + 1) * B_LOC)
        m = dict(shared)
        m["h16"] = np.ascontiguousarray(h[sl]).astype(np.float16)
        m["qpb"] = qpb[sl].astype(np.float16)
        m["aembT"] = np.ascontiguousarray(a_emb[sl].T).astype(np.float16)
        in_maps.append(m)
    return in_maps, apply_gb, apply_rngb


def kernel(**inputs):
    _patch_toolchain()
    from concourse.bass_utils import run_bass_kernel_spmd

    in_maps, apply_gb, apply_rngb = _host_prep(inputs)
    use_native_mish = os.environ.get("KMISH", "0") == "1"
    key = (use_native_mish, apply_gb, apply_rngb)
    if key not in _CACHED:
        _CACHED[key] = _build_bass(*key)
    nc = _CACHED[key]

    res = run_bass_kernel_spmd(nc, in_maps, core_ids=list(range(N_CORES)))
    h_scaled = np.concatenate([r["h_out"] for r in res.results], 0)
    logits = np.concatenate([r["lg_out"] for r in res.results], 0)
    return h_scaled.astype(np.float32), logits.astype(np.float32)


# revision 23
# speedup vs baseline: 1.0302x; 1.0302x over previous
"""Bass/Trainium2 kernel for nn_DynamicsNet (gnn_message_passing).

Sharding: pure data parallel over batch, 8 cores x 256 samples.

Device design (per core, B_LOC=256 samples, chunks of CH=16):
  - Activations in SBUF fp16; PSUM accumulates fp32.
  - Two per-sample layouts:
      nm (node-major):    [n=96 partitions, d=128 free]  -> LN / mish / residual
      fm (feature-major): [d=128 partitions, n=96 free]  -> conv intermediate
  - A-step (graph conv):  out_b = x_b.T @ A        per-sample matmul, nm -> fm
  - W-step (1x1 conv):    out_b = msg_b.T @ w.T    per-sample matmul, fm -> nm
    (the layout flip rides the matmul: the activation is the stationary operand)
  - conv bias: added during PSUM->SBUF evacuation (TT add, bcast bias tile)
  - LN stats: chunked 3D bn_stats, manual even/odd aggregation, half-layer
    (128-sample) granularity for the tiny derive ops.
  - mish: Exp -> Ln(1+x) -> Tanh ACT chain + DVE mul (native Mish/Softplus
    LUTs are broken on this toolchain).
  - final: per-sample PE transpose (nm -> output [d, n]) + min-max scaling
    fused into the PSUM evacuation; min/max via DVE reduce + GPSIMD
    partition_all_reduce.
"""

import os

import numpy as np

N_NODES, D_MODEL, N_BLOCKS = 96, 128, 8
B_FULL = 2048
N_CORES = 8
B_LOC = B_FULL // N_CORES          # 256
CH = 16                            # samples per chunk
N_CH = B_LOC // CH                 # 16 chunks
HALF = B_LOC // 2                  # 128 samples per stats half
LOGITS = 401
EPS = 1e-5

_CACHED = {}
_PATCHED = [False]


def _patch_toolchain():
    """One-time environment fixups (idempotent):

    1. walrus in this container encodes at most ONE sync-wait per ISA
       instruction ("Too many sync wait commands"); Tile emits multi-wait
       instructions. Legalize at BIR-serialization time by hoisting extra
       waits into preceding same-engine NoOps (engine FIFO makes this
       semantically identical).
    2. The PWP act tables ship a mish LUT but act_info.json omits "mish"
       from the mish_and_others set, so lower_act rejects
       ActivationFunctionType.Mish. Patch the json (nix store is writable
       here); also point BASS_ACT_ROOT_JSON_PATH at it for the direct path.
    """
    if _PATCHED[0]:
        return
    _PATCHED[0] = True

    import json as _json

    import concourse.bass as bass

    orig = bass.Bass.to_json_bytes

    def to_json_bytes_legal(self, *a, **k):
        d = _json.loads(orig(self, *a, **k))
        ctr = 0
        for fn in d.get("functions", []):
            for blk in fn.get("blocks", []):
                out = []
                for inst in blk.get("instructions", []):
                    si = inst.get("sync_info")
                    if si:
                        waits = si.get("on_wait") or []
                        if len(waits) > 1:
                            for w in waits[:-1]:
                                ctr += 1
                                out.append({
                                    "debug": inst.get("debug", 0),
                                    "engine": inst["engine"],
                                    "ins": [], "outs": [],
                                    "name": f"I-syncw{ctr}",
                                    "opcode": "NoOp",
                                    "sync_info": {"on_wait": [w],
                                                  "on_update": []},
                                })
                            si["on_wait"] = waits[-1:]
                    out.append(inst)
                blk["instructions"] = out
        return _json.dumps(d).encode()

    bass.Bass.to_json_bytes = to_json_bytes_legal

    # --- act tables: enable mish ---
    try:
        from neuronxcc.driver.Job import Job
        from neuronxcc.driver.jobs.support.FindActInfo import findActInfoFile
        p = findActInfoFile(Job.getPackageDir(), "gen3")
        with open(p) as f:
            d = _json.load(f)
        changed = False
        for s in d.get("act_func_sets", []):
            if s["name"] == "mish_and_others" and "mish" not in s["act"]:
                s["act"]["mish"] = 4
                changed = True
            if s["name"] == "softplus_and_others" and "softplus" not in s["act"]:
                s["act"]["softplus"] = 4
                changed = True
        if changed:
            with open(p, "w") as f:
                _json.dump(d, f)
        os.environ.setdefault("BASS_ACT_ROOT_JSON_PATH", p)
    except Exception:
        pass


def _build_bass(use_native_mish: bool, apply_gb: bool, apply_rngb: bool):
    import concourse.bass as bass
    import concourse.bass_isa as bass_isa
    import concourse.mybir as mybir
    import concourse.tile as tile

    f16 = mybir.dt.float16
    f32 = mybir.dt.float32
    AF = mybir.ActivationFunctionType
    OP = mybir.AluOpType
    AX = mybir.AxisListType

    nc = bass.Bass()

    # ---------------- DRAM tensors ----------------
    h_in = nc.dram_tensor("h16", [B_LOC, D_MODEL, N_NODES], f16, kind="ExternalInput")
    qpb_in = nc.dram_tensor("qpb", [B_LOC, D_MODEL], f16, kind="ExternalInput")
    aembT_in = nc.dram_tensor("aembT", [D_MODEL, B_LOC], f16, kind="ExternalInput")
    A_in = nc.dram_tensor("A96", [N_NODES, N_NODES], f16, kind="ExternalInput")
    I96_in = nc.dram_tensor("I96", [N_NODES, N_NODES], f16, kind="ExternalInput")
    I128_in = nc.dram_tensor("I128", [128, 128], f16, kind="ExternalInput")
    wAT_in = nc.dram_tensor("wAT", [D_MODEL, D_MODEL], f16, kind="ExternalInput")
    Vt_in = nc.dram_tensor("Vt", [16, D_MODEL, D_MODEL], f16, kind="ExternalInput")
    cbb_in = nc.dram_tensor("cbb", [16, D_MODEL], f16, kind="ExternalInput")
    lng_in = nc.dram_tensor("lng", [16, D_MODEL], f16, kind="ExternalInput")
    lnb_in = nc.dram_tensor("lnb", [16, D_MODEL], f16, kind="ExternalInput")
    r1wT0_in = nc.dram_tensor("r1wT0", [D_MODEL, 64], f16, kind="ExternalInput")
    r1wT1_in = nc.dram_tensor("r1wT1", [D_MODEL, 64], f16, kind="ExternalInput")
    r1b_in = nc.dram_tensor("r1b", [1, 64], f16, kind="ExternalInput")
    r2wT_in = nc.dram_tensor("r2wT", [64, LOGITS], f16, kind="ExternalInput")
    r2b_in = nc.dram_tensor("r2b", [1, LOGITS], f16, kind="ExternalInput")
    rng_in = nc.dram_tensor("rng", [1, 64], f16, kind="ExternalInput")
    rnb_in = nc.dram_tensor("rnb", [1, 64], f16, kind="ExternalInput")

    h_out = nc.dram_tensor("h_out", [B_LOC, D_MODEL, N_NODES], f32,
                           kind="ExternalOutput")
    lg_out = nc.dram_tensor("lg_out", [B_LOC, LOGITS], f32, kind="ExternalOutput")

    SQ128 = float(np.sqrt(128.0))

    def bcast_row(dram_ap, parts, free):
        """AP reading one DRAM row broadcast across `parts` partitions."""
        return bass.AP(tensor=dram_ap.tensor, offset=dram_ap.offset,
                       ap=[[0, parts], [1, free]])

    with tile.TileContext(nc) as tc:
        with (
            tc.tile_pool(name="const", bufs=1) as cpool,
            tc.tile_pool(name="xres", bufs=1) as xpool,
            tc.tile_pool(name="zbuf", bufs=8) as zpool,
            tc.tile_pool(name="htile", bufs=1) as hpool,
            tc.tile_pool(name="msg", bufs=2) as mpool,
            tc.tile_pool(name="ubuf", bufs=4) as upool,
            tc.tile_pool(name="ybuf", bufs=5) as ypool,
            tc.tile_pool(name="stats", bufs=1) as spool,
            tc.tile_pool(name="small", bufs=2) as smpool,
            tc.tile_pool(name="once", bufs=1) as onpool,
            tc.tile_pool(name="ostage", bufs=1) as opool,
            tc.tile_pool(name="dscr", bufs=1, space="DRAM") as dpool,
            tc.tile_pool(name="psA", bufs=1, space="PSUM") as psA,
            tc.tile_pool(name="psW", bufs=1, space="PSUM") as psW,
        ):
            # ------------- constants -------------
            A_sb = cpool.tile([N_NODES, N_NODES], f16)
            nc.sync.dma_start(out=A_sb, in_=A_in[:, :])
            I96_sb = cpool.tile([N_NODES, N_NODES], f16)
            nc.sync.dma_start(out=I96_sb, in_=I96_in[:, :])
            I128_sb = cpool.tile([128, 128], f16)
            nc.sync.dma_start(out=I128_sb, in_=I128_in[:, :])
            wAT_sb = cpool.tile([D_MODEL, D_MODEL], f16)
            nc.sync.dma_start(out=wAT_sb, in_=wAT_in[:, :])
            V_sb = cpool.tile([D_MODEL, 16, D_MODEL], f16)
            for li in range(16):
                nc.sync.dma_start(out=V_sb[:, li, :], in_=Vt_in[li, :, :])
            cb_sb = cpool.tile([N_NODES, 16, D_MODEL], f16)
            for li in range(16):
                nc.sync.dma_start(out=cb_sb[:, li, :],
                                  in_=bcast_row(cbb_in[li, :], N_NODES, D_MODEL))
            if apply_gb:
                g_sb = cpool.tile([N_NODES, 16, D_MODEL], f16)
                b_sb = cpool.tile([N_NODES, 16, D_MODEL], f16)
                for li in range(16):
                    nc.sync.dma_start(out=g_sb[:, li, :],
                                      in_=bcast_row(lng_in[li, :], N_NODES, D_MODEL))
                    nc.sync.dma_start(out=b_sb[:, li, :],
                                      in_=bcast_row(lnb_in[li, :], N_NODES, D_MODEL))

            aembT_sb = cpool.tile([D_MODEL, B_LOC], f16)
            nc.sync.dma_start(out=aembT_sb, in_=aembT_in[:, :])
            r1wT0_sb = cpool.tile([D_MODEL, 64], f16)
            nc.sync.dma_start(out=r1wT0_sb, in_=r1wT0_in[:, :])
            r1wT1_sb = cpool.tile([D_MODEL, 64], f16)
            nc.sync.dma_start(out=r1wT1_sb, in_=r1wT1_in[:, :])
            r1b_sb = cpool.tile([1, 64], f16)
            nc.sync.dma_start(out=r1b_sb, in_=r1b_in[:, :])
            r2wT_sb = cpool.tile([64, LOGITS], f16)
            nc.sync.dma_start(out=r2wT_sb, in_=r2wT_in[:, :])
            r2b_sb = cpool.tile([1, LOGITS], f16)
            nc.sync.dma_start(out=r2b_sb, in_=r2b_in[:, :])
            if apply_rngb:
                rng_sb = cpool.tile([128, 64], f16)
                rnb_sb = cpool.tile([128, 64], f16)
                nc.sync.dma_start(out=rng_sb, in_=bcast_row(rng_in[0, :], 128, 64))
                nc.sync.dma_start(out=rnb_sb, in_=bcast_row(rnb_in[0, :], 128, 64))
            ones_sb = cpool.tile([1, B_LOC], f16)
            nc.vector.memset(ones_sb, 1.0)
            eps_sb = cpool.tile([N_NODES, 1], f32)
            nc.vector.memset(eps_sb, 128.0 * EPS)
            eps64_sb = cpool.tile([128, 1], f32)
            nc.vector.memset(eps64_sb, 64.0 * EPS)
            one96_sb = cpool.tile([N_NODES, 1], f32)
            nc.vector.memset(one96_sb, 1.0)

            x_sb = xpool.tile([N_NODES, B_LOC, D_MODEL], f16)
            hsumf = cpool.tile([D_MODEL, B_LOC], f32)
            hsum16 = cpool.tile([D_MODEL, B_LOC], f16)
            pmax_sb = cpool.tile([N_NODES, B_LOC], f16)
            pmin_sb = cpool.tile([N_NODES, B_LOC], f16)

            # ---------------- proj ----------------
            for c in range(N_CH):
                b0 = c * CH
                ht = hpool.tile([D_MODEL, CH, N_NODES], f16, tag="ht")
                nc.sync.dma_start(
                    out=ht, in_=h_in[b0:b0 + CH, :, :].rearrange("b c n -> c b n"))
                nc.vector.tensor_reduce(out=hsumf[:, b0:b0 + CH], in_=ht,
                                        axis=AX.X, op=OP.add)
                # qpb rows broadcast across the 96 node partitions
                qsl = qpb_in[b0:b0 + CH, :]
                qb = mpool.tile([N_NODES, CH, D_MODEL], f16, tag="msg")
                nc.sync.dma_start(
                    out=qb,
                    in_=bass.AP(tensor=qsl.tensor, offset=qsl.offset,
                                ap=[[0, N_NODES], [D_MODEL, CH], [1, D_MODEL]]))
                psx = psW.tile([N_NODES, CH, D_MODEL], f32, tag="W")
                for j in range(CH):
                    nc.tensor.matmul(psx[:, j, :], ht[:, j, :], wAT_sb,
                                     start=True, stop=True)
                nc.vector.tensor_tensor(out=x_sb[:, b0:b0 + CH, :], in0=psx,
                                        in1=qb, op=OP.add)
            nc.vector.tensor_copy(hsum16, hsumf)

            # ---------------- blocks ----------------
            def half_block(l_idx, src_of, with_mish, y_chunks):
                """src nm -> A -> W(+bias) -> LN (+mish into y | +residual x).

                src_of(c) -> [96, CH, 128] tile/AP for chunk c.
                with_mish: writes mish output into fresh y_chunks[c] tiles.
                else: residual-adds into x_sb.
                """
                for hf in range(2):
                    zch = [None] * (N_CH // 2)
                    stats6 = spool.tile([N_NODES, HALF, 6], f32, tag="st6")
                    hb0 = hf * HALF
                    for c in range(N_CH // 2):
                        b0 = hb0 + c * CH
                        zb0 = c * CH
                        srct = src_of(hf * (N_CH // 2) + c)
                        # pad per-sample stride to 512B so each matmul
                        # output stays inside one PSUM bank
                        psa = psA.tile([D_MODEL, CH, 128], f32, tag="A")
                        for j in range(CH):
                            nc.tensor.matmul(psa[:, j, 0:N_NODES], srct[:, j, :],
                                             A_sb, start=True, stop=True)
                        msg = mpool.tile([D_MODEL, CH, N_NODES], f16, tag="msg")
                        nc.scalar.copy(out=msg, in_=psa[:, :, 0:N_NODES])
                        psw = psW.tile([N_NODES, CH, D_MODEL], f32, tag="W")
                        for j in range(CH):
                            nc.tensor.matmul(psw[:, j, :], msg[:, j, :],
                                             V_sb[:, l_idx, :], start=True,
                                             stop=True)
                        cb_bc = bass.AP(
                            tensor=cb_sb.tensor,
                            offset=cb_sb[:, l_idx, :].offset,
                            ap=[list(cb_sb[:, l_idx, :].ap[0]),
                                [0, CH], [1, D_MODEL]])
                        zt = zpool.tile([N_NODES, CH, D_MODEL], f16, tag="z")
                        zch[c] = zt
                        nc.vector.tensor_tensor(out=zt, in0=psw, in1=cb_bc,
                                                op=OP.add)
                        for j in range(CH):  # walrus: bn_stats out = 6/partition
                            nc.vector.bn_stats(out=stats6[:, zb0 + j, :],
                                               in_=zt[:, j, :])
                    # derive mean (mm) and rstd (rr) for this half
                    s1 = stats6[:, :, 1]
                    s2 = stats6[:, :, 2]
                    s4 = stats6[:, :, 4]
                    s5 = stats6[:, :, 5]
                    mm = smpool.tile([N_NODES, HALF], f32, tag="mm")
                    nc.vector.tensor_tensor(out=mm, in0=s1, in1=s4, op=OP.add)
                    nc.vector.tensor_scalar_mul(mm, mm, 0.5)
                    dlt = smpool.tile([N_NODES, HALF], f32, tag="dlt")
                    nc.vector.tensor_tensor(out=dlt, in0=s1, in1=s4,
                                            op=OP.subtract)
                    tv = smpool.tile([N_NODES, HALF], f32, tag="tv")
                    nc.vector.tensor_tensor(out=tv, in0=dlt, in1=dlt, op=OP.mult)
                    nc.vector.scalar_tensor_tensor(out=tv, in0=tv, scalar=32.0,
                                                   in1=s2, op0=OP.mult, op1=OP.add)
                    nc.vector.tensor_tensor(out=tv, in0=tv, in1=s5, op=OP.add)
                    nc.scalar.activation(out=tv, in_=tv, func=AF.Sqrt,
                                         bias=eps_sb, scale=1.0)
                    rr = smpool.tile([N_NODES, HALF], f32, tag="rr")
                    nc.vector.reciprocal(out=rr, in_=tv)
                    nc.vector.tensor_scalar_mul(rr, rr, SQ128)
                    # apply (+ mish / + residual), mish ACT funcs batched
                    # over 4-chunk quarters to amortize ACT table-set loads
                    QC = 4
                    for q in range(0, N_CH // 2, QC):
                        us = []
                        for c in range(q, q + QC):
                            b0 = hf * HALF + c * CH
                            zb0 = c * CH
                            u = upool.tile([N_NODES, CH, D_MODEL], f16, tag="u")
                            us.append(u)
                            zt = zch[c]
                            for j in range(CH):
                                zb = zb0 + j
                                nc.vector.tensor_scalar(
                                    out=u[:, j, :], in0=zt[:, j, :],
                                    scalar1=mm[:, zb:zb + 1],
                                    scalar2=rr[:, zb:zb + 1],
                                    op0=OP.subtract, op1=OP.mult)
                            if apply_gb:
                                g_bc = bass.AP(
                                    tensor=g_sb.tensor,
                                    offset=g_sb[:, l_idx, :].offset,
                                    ap=[list(g_sb[:, l_idx, :].ap[0]),
                                        [0, CH], [1, D_MODEL]])
                                b_bc = bass.AP(
                                    tensor=b_sb.tensor,
                                    offset=b_sb[:, l_idx, :].offset,
                                    ap=[list(b_sb[:, l_idx, :].ap[0]),
                                        [0, CH], [1, D_MODEL]])
                                nc.vector.tensor_tensor(out=u, in0=u, in1=g_bc,
                                                        op=OP.mult)
                                nc.vector.tensor_tensor(out=u, in0=u, in1=b_bc,
                                                        op=OP.add)
                        if with_mish:
                            ews = []
                            for i, c in enumerate(range(q, q + QC)):
                                ew = upool.tile([N_NODES, CH, D_MODEL], f16,
                                                tag="ew")
                                ews.append(ew)
                                nc.scalar.activation(out=ew, in_=us[i],
                                                     func=AF.Exp)
                            for ew in ews:
                                nc.scalar.activation(out=ew, in_=ew, func=AF.Ln,
                                                     bias=one96_sb, scale=1.0)
                            for ew in ews:
                                nc.scalar.activation(out=ew, in_=ew,
                                                     func=AF.Tanh)
                            for i, c in enumerate(range(q, q + QC)):
                                yt = ypool.tile([N_NODES, CH, D_MODEL], f16,
                                                tag="y")
                                y_chunks[hf * (N_CH // 2) + c] = yt
                                nc.vector.tensor_tensor(out=yt, in0=us[i],
                                                        in1=ews[i], op=OP.mult)
                        else:
                            for i, c in enumerate(range(q, q + QC)):
                                b0 = hf * HALF + c * CH
                                nc.gpsimd.tensor_tensor(
                                    out=x_sb[:, b0:b0 + CH, :],
                                    in0=x_sb[:, b0:b0 + CH, :], in1=us[i],
                                    op=OP.add)

            def x_of(c):
                return x_sb[:, c * CH:(c + 1) * CH, :]

            for blk in range(N_BLOCKS):
                y_chunks = [None] * N_CH
                half_block(2 * blk + 0, x_of, True, y_chunks)
                half_block(2 * blk + 1, lambda c: y_chunks[c], False, None)

            # ---------------- min/max + final transpose/scale ----------------
            for c in range(N_CH):
                b0 = c * CH
                nc.vector.tensor_reduce(out=pmax_sb[:, b0:b0 + CH],
                                        in_=x_sb[:, b0:b0 + CH, :], axis=AX.X,
                                        op=OP.max)
                nc.vector.tensor_reduce(out=pmin_sb[:, b0:b0 + CH],
                                        in_=x_sb[:, b0:b0 + CH, :], axis=AX.X,
                                        op=OP.min)
            # cross-partition (96 -> 1) min/max via PE transposes
            mxT = onpool.tile([128, 2], f16, tag="mxT")
            mnT = onpool.tile([128, 2], f16, tag="mnT")
            for hf in range(2):
                psT = psW.tile([128, N_NODES], f16, tag="W")
                nc.tensor.matmul(psT, pmax_sb[:, hf * 128:(hf + 1) * 128],
                                 I96_sb, start=True, stop=True, is_transpose=True)
                nc.vector.tensor_reduce(out=mxT[:, hf:hf + 1], in_=psT,
                                        axis=AX.X, op=OP.max)
                psT2 = psW.tile([128, N_NODES], f16, tag="W")
                nc.tensor.matmul(psT2, pmin_sb[:, hf * 128:(hf + 1) * 128],
                                 I96_sb, start=True, stop=True, is_transpose=True)
                nc.vector.tensor_reduce(out=mnT[:, hf:hf + 1], in_=psT2,
                                        axis=AX.X, op=OP.min)
            # transpose [128, 2] -> [2, 128], evac, DMA-broadcast to all parts
            ps2a = psW.tile([2, 128], f16, tag="W")
            nc.tensor.matmul(ps2a, mxT, I128_sb, start=True, stop=True,
                             is_transpose=True)
            mx1p = onpool.tile([2, 128], f16, tag="mx1p")
            nc.vector.tensor_copy(mx1p, ps2a)
            ps2b = psW.tile([2, 128], f16, tag="W")
            nc.tensor.matmul(ps2b, mnT, I128_sb, start=True, stop=True,
                             is_transpose=True)
            mn1p = onpool.tile([2, 128], f16, tag="mn1p")
            nc.vector.tensor_copy(mn1p, ps2b)
            mxd = dpool.tile([2, 128], f16, tag="mxd")
            mnd = dpool.tile([2, 128], f16, tag="mnd")
            nc.sync.dma_start(out=mxd, in_=mx1p)
            nc.sync.dma_start(out=mnd, in_=mn1p)
            mx_b16 = onpool.tile([128, 2, 128], f16, tag="mxa")
            mn_b16 = onpool.tile([128, 2, 128], f16, tag="mna")
            for j in range(2):
                nc.sync.dma_start(out=mx_b16[:, j, :],
                                  in_=bcast_row(mxd[j, :], 128, 128))
                nc.sync.dma_start(out=mn_b16[:, j, :],
                                  in_=bcast_row(mnd[j, :], 128, 128))
            mn_all = onpool.tile([128, B_LOC], f32, tag="mnf")
            nc.vector.tensor_copy(mn_all, mn_b16.rearrange("p a b -> p (a b)"))
            sc = onpool.tile([128, B_LOC], f32, tag="sc")
            nc.vector.tensor_tensor(out=sc,
                                    in0=mx_b16.rearrange("p a b -> p (a b)"),
                                    in1=mn_all, op=OP.subtract)
            lt = onpool.tile([128, B_LOC], f32, tag="lt")
            nc.vector.tensor_scalar(out=lt, in0=sc, scalar1=1e-5, scalar2=1e-5,
                                    op0=OP.is_lt, op1=OP.mult)
            nc.vector.tensor_tensor(out=sc, in0=sc, in1=lt, op=OP.add)
            rs_all = onpool.tile([128, B_LOC], f32, tag="rsa")
            nc.vector.reciprocal(out=rs_all, in_=sc)

            for c in range(N_CH):
                b0 = c * CH
                pso = psA.tile([D_MODEL, CH, 128], f16, tag="A")
                for j in range(CH):
                    nc.tensor.matmul(pso[:, j, 0:N_NODES], x_sb[:, b0 + j, :],
                                     I96_sb, start=True, stop=True,
                                     is_transpose=True)
                ost = opool.tile([D_MODEL, CH, N_NODES], f32, tag="ost")
                for j in range(CH):
                    b = b0 + j
                    nc.vector.tensor_scalar(
                        out=ost[:, j, :], in0=pso[:, j, 0:N_NODES],
                        scalar1=mn_all[:, b:b + 1], scalar2=rs_all[:, b:b + 1],
                        op0=OP.subtract, op1=OP.mult)
                nc.sync.dma_start(
                    out=h_out[b0:b0 + CH, :, :].rearrange("b c n -> c b n"),
                    in_=ost)

            # ---------------- reward head ----------------
            psr = psW.tile([64, B_LOC], f32, tag="W")
            nc.tensor.matmul(psr, r1wT0_sb, hsum16, start=True, stop=False)
            nc.tensor.matmul(psr, r1wT1_sb, aembT_sb, start=False, stop=False)
            nc.tensor.matmul(psr, r1b_sb, ones_sb, start=False, stop=True)
            rIn = onpool.tile([64, B_LOC], f16, tag="rin")
            nc.scalar.copy(out=rIn, in_=psr)
            rT = onpool.tile([128, 2, 64], f16, tag="rT")
            for hf in range(2):
                pst = psW.tile([128, 64], f16, tag="W")
                nc.tensor.matmul(pst, rIn[:, hf * 128:(hf + 1) * 128],
                                 I128_sb[0:64, 0:64], start=True, stop=True,
                                 is_transpose=True)
                nc.vector.tensor_copy(rT[:, hf, :], pst)
            rst6 = onpool.tile([128, 2, 6], f32, tag="rst6")
            for hf in range(2):
                nc.vector.bn_stats(out=rst6[:, hf, :], in_=rT[:, hf, :])
            rmm = onpool.tile([128, 2], f32, tag="rmm")
            nc.vector.tensor_tensor(out=rmm, in0=rst6[:, :, 1], in1=rst6[:, :, 4],
                                    op=OP.add)
            nc.vector.tensor_scalar_mul(rmm, rmm, 0.5)
            rdl = onpool.tile([128, 2], f32, tag="rdl")
            nc.vector.tensor_tensor(out=rdl, in0=rst6[:, :, 1], in1=rst6[:, :, 4],
                                    op=OP.subtract)
            rtv = onpool.tile([128, 2], f32, tag="rtv")
            nc.vector.tensor_tensor(out=rtv, in0=rdl, in1=rdl, op=OP.mult)
            nc.vector.scalar_tensor_tensor(out=rtv, in0=rtv, scalar=16.0,
                                           in1=rst6[:, :, 2], op0=OP.mult,
                                           op1=OP.add)
            nc.vector.tensor_tensor(out=rtv, in0=rtv, in1=rst6[:, :, 5], op=OP.add)
            nc.scalar.activation(out=rtv, in_=rtv, func=AF.Sqrt, bias=eps64_sb,
                                 scale=1.0)
            rrr = onpool.tile([128, 2], f32, tag="rrr")
            nc.vector.reciprocal(out=rrr, in_=rtv)
            nc.vector.tensor_scalar_mul(rrr, rrr, 8.0)
            for hf in range(2):
                nc.vector.tensor_scalar(
                    out=rT[:, hf, :], in0=rT[:, hf, :],
                    scalar1=rmm[:, hf:hf + 1], scalar2=rrr[:, hf:hf + 1],
                    op0=OP.subtract, op1=OP.mult)
                if apply_rngb:
                    nc.vector.tensor_tensor(out=rT[:, hf, :], in0=rT[:, hf, :],
                                            in1=rng_sb, op=OP.mult)
                    nc.vector.tensor_tensor(out=rT[:, hf, :], in0=rT[:, hf, :],
                                            in1=rnb_sb, op=OP.add)
            if use_native_mish:
                nc.scalar.activation(out=rT, in_=rT, func=AF.Mish)
            else:
                rew = onpool.tile([128, 2, 64], f16, tag="rew")
                rone = onpool.tile([128, 1], f32, tag="rone")
                nc.vector.memset(rone, 1.0)
                nc.scalar.activation(out=rew, in_=rT, func=AF.Exp)
                nc.scalar.activation(out=rew, in_=rew, func=AF.Ln, bias=rone,
                                     scale=1.0)
                nc.scalar.activation(out=rew, in_=rew, func=AF.Tanh)
                nc.vector.tensor_tensor(out=rT, in0=rT, in1=rew, op=OP.mult)
            lstage = onpool.tile([128, 2, LOGITS], f32, tag="lst")
            for hf in range(2):
                pstb = psW.tile([64, 128], f16, tag="W")
                nc.tensor.matmul(pstb, rT[:, hf, :], I128_sb, start=True,
                                 stop=True, is_transpose=True)
                rK = onpool.tile([64, 128], f16, tag="rK")
                nc.vector.tensor_copy(rK, pstb)
                psl = psW.tile([128, LOGITS], f32, tag="W")
                nc.tensor.matmul(psl, rK, r2wT_sb, start=True, stop=False)
                nc.tensor.matmul(psl, ones_sb[:, 0:128], r2b_sb, start=False,
                                 stop=True)
                nc.scalar.copy(out=lstage[:, hf, :], in_=psl)
                nc.sync.dma_start(out=lg_out[hf * 128:(hf + 1) * 128, :],
                                  in_=lstage[:, hf, :])

    return nc


def _host_prep(inputs):
    h = np.asarray(inputs["h"], np.float32)
    a = np.asarray(inputs["a"]).astype(np.int64)
    piece_id = np.asarray(inputs["piece_id"]).astype(np.int64)
    A_norm = np.asarray(inputs["A_norm"], np.float32)
    piece_emb = np.asarray(inputs["piece_emb"], np.float32)
    pos_emb = np.asarray(inputs["pos_emb"], np.float32)
    proj_w = np.asarray(inputs["proj_w"], np.float32)
    proj_b = np.asarray(inputs["proj_b"], np.float32)
    conv_w = np.asarray(inputs["conv_w"], np.float32)
    conv_b = np.asarray(inputs["conv_b"], np.float32)
    ln_g = np.asarray(inputs["ln_g"], np.float32)
    ln_b = np.asarray(inputs["ln_b"], np.float32)
    r1_w = np.asarray(inputs["r1_w"], np.float32)
    r1_b = np.asarray(inputs["r1_b"], np.float32)
    rn_g = np.asarray(inputs["rn_g"], np.float32)
    rn_b = np.asarray(inputs["rn_b"], np.float32)
    r2_w = np.asarray(inputs["r2_w"], np.float32)
    r2_b = np.asarray(inputs["r2_b"], np.float32)

    a_emb = piece_emb[piece_id] + pos_emb[a % N_NODES]
    qpb = a_emb @ proj_w[:, D_MODEL:].T + proj_b

    apply_gb = not (np.allclose(ln_g, 1.0) and np.allclose(ln_b, 0.0))
    apply_rngb = not (np.allclose(rn_g, 1.0) and np.allclose(rn_b, 0.0))

    shared = {
        "A96": A_norm.astype(np.float16),
        "I96": np.eye(N_NODES, dtype=np.float16),
        "I128": np.eye(128, dtype=np.float16),
        "wAT": np.ascontiguousarray(proj_w[:, :D_MODEL].T).astype(np.float16),
        "Vt": np.ascontiguousarray(
            conv_w.reshape(16, D_MODEL, D_MODEL).swapaxes(1, 2)).astype(np.float16),
        "cbb": conv_b.reshape(16, D_MODEL).astype(np.float16),
        "lng": ln_g.reshape(16, D_MODEL).astype(np.float16),
        "lnb": ln_b.reshape(16, D_MODEL).astype(np.float16),
        "r1wT0": np.ascontiguousarray(
            (r1_w[:, :D_MODEL] / float(N_NODES)).T).astype(np.float16),
        "r1wT1": np.ascontiguousarray(r1_w[:, D_MODEL:].T).astype(np.float16),
        "r1b": r1_b.reshape(1, 64).astype(np.float16),
        "r2wT": np.ascontiguousarray(r2_w.T).astype(np.float16),
        "r2b": r2_b.reshape(1, LOGITS).astype(np.float16),
        "rng": rn_g.reshape(1, 64).astype(np.float16),
        "rnb": rn_b.reshape(1, 64).astype(np.float16),
    }
    in_maps = []
    for s in range(N_CORES):
        sl = slice(s * B_LOC, (s + 1) * B_LOC)
        m = dict(shared)
        m["h16"] = np.ascontiguousarray(h[sl]).astype(np.float16)
        m["qpb"] = qpb[sl].astype(np.float16)
        m["aembT"] = np.ascontiguousarray(a_emb[sl].T).astype(np.float16)
        in_maps.append(m)
    return in_maps, apply_gb, apply_rngb


def kernel(**inputs):
    _patch_toolchain()
    from concourse.bass_utils import run_bass_kernel_spmd

    in_maps, apply_gb, apply_rngb = _host_prep(inputs)
    use_native_mish = os.environ.get("KMISH", "0") == "1"
    key = (use_native_mish, apply_gb, apply_rngb)
    if key not in _CACHED:
        _CACHED[key] = _build_bass(*key)
    nc = _CACHED[key]

    res = run_bass_kernel_spmd(nc, in_maps, core_ids=list(range(N_CORES)))
    h_scaled = np.concatenate([r["h_out"] for r in res.results], 0)
    logits = np.concatenate([r["lg_out"] for r in res.results], 0)
    return h_scaled.astype(np.float32), logits.astype(np.float32)
